# revision 1
# baseline (speedup 1.0000x reference)
"""Trainium2 Bass kernel for Linformer self-attention (ragged projection).

Reference computation (per batch sample b, data-parallel over 8 cores):
    L      = sum(mask > -1)                      # valid length
    hk     = h @ Wk.T + bk                       # [S, D]
    hv     = h @ Wv.T + bv
    mm[s]  = (mask[s] > -1) / sqrt(L)
    kT     = hk.T @ (pk * mm[:, None])           # [D, K]   (= hkp.T)
    v      = (pv * mm[:, None]).T @ hv           # [K, D]   (= hvp)
    q      = (h @ Wq.T + bq) * DH**-0.5          # via qT [D, S]
    per head i (rows 64i:64i+64 of qT/kT, cols of v):
        scoresT = k_i @ q_i.T                    # [K, S] chunks
        probsT  = exp(scoresT)                   # un-normalized, bf16
        ctx_i   = (probsT.T @ [v_i | 1]) ; ctx_i = num / den
    out[s, 64i+dh] = ctx_i[s, dh]

Layouts chosen so every matmul contracts over the SBUF partition dim and
every DRAM access is partition-major contiguous. h and W are transposed
host-side (pure layout prep); everything else runs on device.

Matmuls run as float32r (TF32-like, full PE rate at free dim >= 256)
except the ctx matmul which runs bf16 (probs/v), accumulating fp32.
"""

import numpy as np

import concourse.mybir as mybir
import concourse.tile as tile
from concourse import bacc
from concourse import bass_utils

P = 128
f32 = mybir.dt.float32
f32r = mybir.dt.float32r
bf16 = mybir.dt.bfloat16
AF = mybir.ActivationFunctionType
ALU = mybir.AluOpType

# Problem dims (nn_LinformerSelfAttention): B=8, S=4096, D=1024, H=16, K=256
B = 8
S_FULL = 4096
D_FULL = 1024
KL_FULL = 256
DH = 64


def build_program(S=S_FULL, D=D_FULL, KL=KL_FULL):
    """Emit the per-core Bass program. Returns compiled Bacc."""
    SC = S // P          # s-chunks of 128
    SG = S // 512        # s-groups of 512
    DC = D // P          # d-chunks of 128
    D5 = D // 512        # d-halves of 512
    KC = KL // P         # linformer-k chunks of 128
    H = D // DH          # heads
    HPM = P // DH        # heads per 128-partition m-tile (2)
    assert S % 512 == 0 and D % 512 == 0 and KL % P == 0
    assert 512 % KL == 0 or KL % 512 == 0

    nc = bacc.Bacc("TRN2", target_bir_lowering=False, debug=False)

    hT = nc.dram_tensor("hT", [D, S], f32r, kind="ExternalInput")
    msk = nc.dram_tensor("mask", [S], f32, kind="ExternalInput")
    wqT = nc.dram_tensor("wqT", [D, D], f32r, kind="ExternalInput")
    wkT = nc.dram_tensor("wkT", [D, D], f32r, kind="ExternalInput")
    wvT = nc.dram_tensor("wvT", [D, D], f32r, kind="ExternalInput")
    bq = nc.dram_tensor("bq", [D], f32, kind="ExternalInput")
    bk = nc.dram_tensor("bk", [D], f32, kind="ExternalInput")
    bv = nc.dram_tensor("bv", [D], f32, kind="ExternalInput")
    pk = nc.dram_tensor("pk", [S, KL], f32r, kind="ExternalInput")
    pv = nc.dram_tensor("pv", [S, KL], f32r, kind="ExternalInput")
    out = nc.dram_tensor("out", [S, D], f32, kind="ExternalOutput")

    with tile.TileContext(nc) as tc:
        with (
            tc.tile_pool(name="persist", bufs=1) as persist,
            tc.tile_pool(name="wpool", bufs=1) as wpool,
            tc.tile_pool(name="hpool", bufs=2) as hpool,
            tc.tile_pool(name="spool", bufs=2) as spool,
            tc.tile_pool(name="hkpool", bufs=2) as hkpool,
            tc.tile_pool(name="cpool", bufs=1) as cpool,
        ):
            # ---------- setup: mask stats ----------
            with tc.tile_pool(name="psetup", bufs=1, space="PSUM") as psetup:
                mt = spool.tile([P, SC], f32, tag="mt")
                nc.sync.dma_start(out=mt[:], in_=msk.ap().rearrange("(c p) -> p c", p=P))
                m01 = spool.tile([P, SC], f32, tag="m01")
                nc.vector.tensor_scalar(m01[:], mt[:], -1.0, None, ALU.is_gt)
                lp = spool.tile([P, 1], f32, tag="lp")
                nc.vector.tensor_reduce(lp[:], m01[:], mybir.AxisListType.X, ALU.add)
                ones_col = spool.tile([P, 1], f32, tag="onc")
                nc.vector.memset(ones_col[:], 1.0)
                ones_row = spool.tile([1, P], f32, tag="onr")
                nc.vector.memset(ones_row[:], 1.0)
                lps = psetup.tile([1, 1], f32)
                nc.tensor.matmul(lps[:], lp[:], ones_col[:],
                                 start=True, stop=True)
                lrec = spool.tile([1, 1], f32, tag="lrec")
                nc.vector.reciprocal(lrec[:], lps[:])
                inv = spool.tile([1, 1], f32, tag="inv")
                nc.scalar.activation(inv[:], lrec[:], AF.Sqrt)
                invps = psetup.tile([P, 1], f32)
                nc.tensor.matmul(invps[:], ones_row[:],
                                 inv[:], start=True, stop=True)
                invcol = persist.tile([P, 1], f32, tag="invcol")
                nc.vector.tensor_copy(invcol[:], invps[:])
                # mm = (mask > -1) / sqrt(L), per-s column layout [P, SC]
                mm_sb = persist.tile([P, SC], f32, tag="mmsb")
                nc.vector.tensor_scalar(mm_sb[:], m01[:], invcol[:], None, ALU.mult)

            # ---------- setup: biases ----------
            bk_rep = persist.tile([P, D], f32, tag="bkrep")
            nc.gpsimd.dma_start(out=bk_rep[:], in_=bk.ap()[None, :].broadcast_to((P, D)))
            bv_rep = persist.tile([P, D], f32, tag="bvrep")
            nc.gpsimd.dma_start(out=bv_rep[:], in_=bv.ap()[None, :].broadcast_to((P, D)))
            bq_sb = spool.tile([P, DC], f32, tag="bqsb")
            nc.sync.dma_start(out=bq_sb[:], in_=bq.ap().rearrange("(m p) -> p m", p=P))
            bq_scaled = persist.tile([P, DC], f32, tag="bqsc")
            nc.vector.tensor_scalar(bq_scaled[:], bq_sb[:], float(DH) ** -0.5, None,
                                    ALU.mult)

            kt_sb = persist.tile([P, DC, KL], f32r, tag="ktsb")
            vaug = persist.tile([P, H * KC, DH + 1], bf16, tag="vaug")

            # ---------- phase Ik / Iv ----------
            for which in ("k", "v"):
                w_dram = wkT if which == "k" else wvT
                p_dram = pk if which == "k" else pv
                brep = bk_rep if which == "k" else bv_rep
                w_sb = wpool.tile([P, DC, D], f32r, tag="w", name="wsb")
                for d in range(DC):
                    nc.sync.dma_start(out=w_sb[:, d, :],
                                      in_=w_dram.ap()[P * d:P * (d + 1), :])
                with (
                    tc.tile_pool(name="pacc", bufs=1, space="PSUM") as pacc,
                    tc.tile_pool(name="phk", bufs=2, space="PSUM") as phk,
                ):
                    if which == "v":
                        # v accumulators: KC*D5 banks, one group each spanning
                        # the whole s loop
                        acc = [pacc.tile([P, 512], f32, tag=f"acc{j}",
                                         name=f"accv{j}")
                               for j in range(KC * D5)]
                    for g in range(SG):
                        ht_g = hpool.tile([P, DC, 512], f32r, tag="ht")
                        for d in range(DC):
                            nc.sync.dma_start(
                                out=ht_g[:, d, :],
                                in_=hT.ap()[P * d:P * (d + 1), 512 * g:512 * (g + 1)])
                        pkm_g = spool.tile([P, 4, KL], f32r, tag="pkm")
                        hk_g = hkpool.tile([P, 4, D], f32r, tag="hksb")
                        for c in range(4):
                            s = 4 * g + c
                            first, last = s == 0, s == SC - 1
                            pk_c = spool.tile([P, KL], f32r, tag="pkc")
                            nc.sync.dma_start(out=pk_c[:],
                                              in_=p_dram.ap()[P * s:P * (s + 1), :])
                            nc.vector.tensor_tensor(
                                pkm_g[:, c, :], pk_c[:],
                                mm_sb[:, s:s + 1].broadcast_to((P, KL)), ALU.mult)
                            hk_ps = [phk.tile([P, 512], f32, tag=f"hk{j}",
                                              name=f"hkps{j}")
                                     for j in range(D5)]
                            for d in range(DC):
                                lhsT = ht_g[:, d, P * c:P * (c + 1)].bitcast(f32r)
                                for j in range(D5):
                                    nc.tensor.matmul(
                                        hk_ps[j][:], lhsT,
                                        w_sb[:, d, 512 * j:512 * (j + 1)].bitcast(f32r),
                                        start=(d == 0), stop=(d == DC - 1))
                            for j in range(D5):
                                nc.vector.tensor_tensor(
                                    hk_g[:, c, 512 * j:512 * (j + 1)], hk_ps[j][:],
                                    brep[:, 512 * j:512 * (j + 1)], ALU.add)
                            if which == "v":
                                for kc in range(KC):
                                    for j in range(D5):
                                        nc.tensor.matmul(
                                            acc[kc * D5 + j][:],
                                            pkm_g[:, c, P * kc:P * (kc + 1)].bitcast(f32r),
                                            hk_g[:, c, 512 * j:512 * (j + 1)].bitcast(f32r),
                                            start=first, stop=last)
                        if which == "k":
                            # two-level: per-group partial kT in 4 banks
                            # (4 m-tiles at a time), then DVE-add into kt_sb
                            for m in range(DC):
                                part = pacc.tile([P, KL], f32, tag=f"kpart{m % 4}",
                                                 name=f"kpart{m % 4}")
                                for c in range(4):
                                    nc.tensor.matmul(
                                        part[:],
                                        hk_g[:, c, P * m:P * (m + 1)].bitcast(f32r),
                                        pkm_g[:, c, :].bitcast(f32r),
                                        start=(c == 0), stop=(c == 3))
                                if g == 0:
                                    nc.vector.tensor_copy(kt_sb[:, m, :], part[:])
                                else:
                                    nc.vector.tensor_tensor(
                                        kt_sb[:, m, :], kt_sb[:, m, :], part[:],
                                        ALU.add)
                    if which == "v":
                        for i in range(H):
                            j, off = divmod(DH * i, 512)
                            for kc in range(KC):
                                nc.vector.tensor_copy(
                                    vaug[:, i * KC + kc, 0:DH],
                                    acc[kc * D5 + j][:, off:off + DH])
                        nc.vector.memset(vaug[:, :, DH:DH + 1], 1.0)

            # ---------- phase Iq fused with phase II ----------
            w_sb = wpool.tile([P, DC, D], f32r, tag="w")
            for d in range(DC):
                nc.sync.dma_start(out=w_sb[:, d, :], in_=wqT.ap()[P * d:P * (d + 1), :])
            with (
                tc.tile_pool(name="pq", bufs=2, space="PSUM") as pq,
                tc.tile_pool(name="psc", bufs=2, space="PSUM") as psc,
                tc.tile_pool(name="pctx", bufs=2, space="PSUM") as pctx,
            ):
                for g in range(SG):
                    ht_g = hpool.tile([P, DC, 512], f32r, tag="ht")
                    for d in range(DC):
                        nc.sync.dma_start(
                            out=ht_g[:, d, :],
                            in_=hT.ap()[P * d:P * (d + 1), 512 * g:512 * (g + 1)])
                    qt_g = spool.tile([P, DC, 512], f32r, tag="qt")
                    for mq in range(DC):
                        q_ps = pq.tile([P, 512], f32, tag="qps")
                        for d in range(DC):
                            nc.tensor.matmul(
                                q_ps[:],
                                w_sb[:, d, P * mq:P * (mq + 1)].bitcast(f32r),
                                ht_g[:, d, :].bitcast(f32r),
                                start=(d == 0), stop=(d == DC - 1))
                        # (q + bq) * DH^-0.5, bias varies along partitions
                        nc.scalar.activation(qt_g[:, mq, :], q_ps[:], AF.Identity,
                                             bias=bq_scaled[:, mq:mq + 1],
                                             scale=float(DH) ** -0.5)
                    ctx_g = cpool.tile([P, 4, D], f32, tag="ctxg")
                    for mq in range(DC):
                        for hh in range(HPM):
                            i = HPM * mq + hh
                            po = DH * hh
                            sc_ps = [psc.tile([P, 512], f32, tag=f"sc{kc}", name=f"scps{kc}")
                                     for kc in range(KC)]
                            for kc in range(KC):
                                nc.tensor.matmul(
                                    sc_ps[kc][:],
                                    kt_sb[po:po + DH, mq, P * kc:P * (kc + 1)].bitcast(f32r),
                                    qt_g[po:po + DH, mq, :].bitcast(f32r),
                                    start=True, stop=True)
                            probT = spool.tile([P, KC, 512], bf16, tag="probT")
                            for kc in range(KC):
                                nc.scalar.activation(probT[:, kc, :], sc_ps[kc][:],
                                                     AF.Exp)
                            ctx_ps = pctx.tile([P, 512], f32, tag="ctxps")
                            for c in range(4):
                                for kc in range(KC):
                                    nc.tensor.matmul(
                                        ctx_ps[:, 128 * c:128 * c + DH + 1],
                                        probT[:, kc, P * c:P * (c + 1)],
                                        vaug[:, i * KC + kc, :],
                                        start=(kc == 0), stop=(kc == KC - 1))
                            rec4 = spool.tile([P, 4], f32, tag="rec4")
                            nc.vector.reciprocal(rec4[:], ctx_ps[:, DH::128])
                            for c in range(4):
                                nc.scalar.activation(
                                    ctx_g[:, c, DH * i:DH * (i + 1)],
                                    ctx_ps[:, 128 * c:128 * c + DH], AF.Copy,
                                    scale=rec4[:, c:c + 1])
                    for c in range(4):
                        s0 = 512 * g + P * c
                        nc.sync.dma_start(out=out.ap()[s0:s0 + P, :],
                                          in_=ctx_g[:, c, :])

    nc.compile()
    return nc


_PROGRAM_CACHE = {}


def _get_program(S, D, KL):
    key = (S, D, KL)
    if key not in _PROGRAM_CACHE:
        _PROGRAM_CACHE[key] = build_program(S, D, KL)
    return _PROGRAM_CACHE[key]


def make_in_maps(hidden_states, attention_mask, Wq, bq, Wk, bk, Wv, bv,
                 proj_k, proj_v):
    """Host-side layout prep + batch sharding (1 sample per core)."""
    h = np.asarray(hidden_states, dtype=np.float32)
    Bn, S, D = h.shape
    wqT = np.ascontiguousarray(np.asarray(Wq, np.float32).T)
    wkT = np.ascontiguousarray(np.asarray(Wk, np.float32).T)
    wvT = np.ascontiguousarray(np.asarray(Wv, np.float32).T)
    pk = np.ascontiguousarray(np.asarray(proj_k, np.float32)[:S])
    pv = np.ascontiguousarray(np.asarray(proj_v, np.float32)[:S])
    bqn = np.asarray(bq, np.float32)
    bkn = np.asarray(bk, np.float32)
    bvn = np.asarray(bv, np.float32)
    mask = np.asarray(attention_mask, np.float32).reshape(Bn, S)
    in_maps = []
    for b in range(Bn):
        in_maps.append(dict(
            hT=np.ascontiguousarray(h[b].T),
            mask=np.ascontiguousarray(mask[b]),
            wqT=wqT, wkT=wkT, wvT=wvT,
            bq=bqn, bk=bkn, bv=bvn,
            pk=pk, pv=pv,
        ))
    return in_maps


def kernel(hidden_states, attention_mask, Wq, bq, Wk, bk, Wv, bv,
           proj_k, proj_v):
    h = np.asarray(hidden_states, dtype=np.float32)
    Bn, S, D = h.shape
    KL = np.asarray(proj_k).shape[1]
    nc = _get_program(S, D, KL)
    in_maps = make_in_maps(hidden_states, attention_mask, Wq, bq, Wk, bk,
                           Wv, bv, proj_k, proj_v)
    res = bass_utils.run_bass_kernel_spmd(nc, in_maps, core_ids=list(range(Bn)))
    return np.stack([res.results[b]["out"] for b in range(Bn)], axis=0)


def time_kernel(hidden_states, attention_mask, Wq, bq, Wk, bk, Wv, bv,
                proj_k, proj_v, k1=8, k2=40):
    """Estimate per-execution device time via pipelined-dispatch slope:
    build the PJRT executable once, keep inputs device-resident, and
    measure marginal wall time per extra NEFF execution."""
    import time as _time
    import jax
    from jax.sharding import Mesh, PartitionSpec, NamedSharding
    from jax.experimental.shard_map import shard_map
    from concourse import bass2jax
    from concourse.bass2jax import _bass_exec_p, install_neuronx_cc_hook

    h = np.asarray(hidden_states, dtype=np.float32)
    Bn = h.shape[0]
    S, D = h.shape[1], h.shape[2]
    KL = np.asarray(proj_k).shape[1]
    nc = _get_program(S, D, KL)
    in_maps = make_in_maps(hidden_states, attention_mask, Wq, bq, Wk, bk,
                           Wv, bv, proj_k, proj_v)
    install_neuronx_cc_hook()
    partition_name = nc.partition_id_tensor.name if nc.partition_id_tensor else None
    in_names, out_names, out_avals = [], [], []
    for alloc in nc.m.functions[0].allocations:
        if not isinstance(alloc, mybir.MemoryLocationSet):
            continue
        name = alloc.memorylocations[0].name
        if alloc.kind == "ExternalInput":
            if name != partition_name:
                in_names.append(name)
        elif alloc.kind == "ExternalOutput":
            out_names.append(name)
            out_avals.append(jax.core.ShapedArray(
                tuple(alloc.tensor_shape), mybir.dt.np(alloc.dtype)))
    n_params = len(in_names)
    all_in = list(in_names) + list(out_names)
    if partition_name is not None:
        all_in.append(partition_name)

    def _body(*args):
        operands = list(args)
        if partition_name is not None:
            operands.append(bass2jax.partition_id_tensor())
        return tuple(_bass_exec_p.bind(
            *operands, out_avals=tuple(out_avals), in_names=tuple(all_in),
            out_names=tuple(out_names), lowering_input_output_aliases=(),
            sim_require_finite=True, sim_require_nnan=True, nc=nc))

    devices = jax.devices()[:Bn]
    mesh = Mesh(np.asarray(devices), ("core",))
    fn = jax.jit(shard_map(_body, mesh=mesh,
                           in_specs=(PartitionSpec("core"),) * (n_params + len(out_names)),
                           out_specs=(PartitionSpec("core"),) * len(out_names),
                           check_rep=False), keep_unused=True)
    sh = NamedSharding(mesh, PartitionSpec("core"))
    dev_in = [jax.device_put(
        np.concatenate([in_maps[c][nm] for c in range(Bn)], axis=0), sh)
        for nm in in_names]
    zer = [jax.device_put(np.zeros((Bn * a.shape[0], *a.shape[1:]), a.dtype), sh)
           for a in out_avals]
    outs = fn(*dev_in, *zer)
    jax.block_until_ready(outs)

    def run(k):
        t0 = _time.time()
        rs = [fn(*dev_in, *zer) for _ in range(k)]
        jax.block_until_ready(rs)
        return _time.time() - t0

    run(2)  # warm
    t_k1 = min(run(k1) for _ in range(2))
    t_k2 = min(run(k2) for _ in range(2))
    per_exec_s = (t_k2 - t_k1) / (k2 - k1)
    return per_exec_s * 1e9



# revision 2
# speedup vs baseline: 2.4681x; 2.4681x over previous
"""Trainium2 Bass kernel v2 for Linformer self-attention (ragged projection).

Per batch sample b (data-parallel over 8 cores), with K=256 << S=4096 the
k/v projections are reordered to project h down to K rows FIRST:

    L        = sum(mask > -1);  mm[s] = (mask[s] > -1) / sqrt(L)
    pkvm     = [pk | pv] * mm[:, None]                  # [S, 2K]
    hpT      = h.T @ pkvm                               # [D, 2K]   phase 1
    cs       = ones.T @ (pkv * mm)                      # [1, 2K]   (bias csum)
    kT       = Wk.T.T @ hpT[:, :K] + bk x csk           # [D, K]    phase 2k
    v        = hpT[:, K:].T @ Wv.T + csv x bv           # [K, D]    phase 2v
    qT       = (Wq.T/sqrt(DH)).T @ h.T + bq/sqrt(DH)    # [D, S]    per group
    per head i:  scT = kT_i.T @ qT_i    [K, 512]        # heads row-packed x2
                 probT = exp(scT)  (unnormalized, bf16)
                 ctx[s, 64i+j] = probT.T @ [v_i | 1]; normalize by col 64

This cuts matmul flops ~1.9x vs computing full [S,D] hk/hv. All matmul
inputs are bf16 (f32 PSUM accumulation); DMA volume is halved vs f32.
ctx is computed in [s, d] layout (probT chunks as the stationary operand)
so the softmax denominator lands per-partition -> cheap tensor_scalar
normalization and contiguous output DMA.
"""

import numpy as np

import concourse.mybir as mybir
import concourse.tile as tile
from concourse import bacc
from concourse import bass_utils

P = 128
f32 = mybir.dt.float32
bf16 = mybir.dt.bfloat16
AF = mybir.ActivationFunctionType
ALU = mybir.AluOpType

# Problem dims (nn_LinformerSelfAttention): B=8, S=4096, D=1024, H=16, K=256
B = 8
S_FULL = 4096
D_FULL = 1024
KL_FULL = 256
DH = 64


def build_program(S=S_FULL, D=D_FULL, KL=KL_FULL):
    SC = S // P           # s-chunks of 128 (32)
    SG = S // 512         # s-groups of 512 (8)
    DC = D // P           # d-chunks of 128 (8)
    KC = KL // P          # linformer-k chunks of 128 (2)
    KV = 2 * KL           # fused [pk|pv] width (512)
    H = D // DH           # heads (16)
    HP = H // 2           # head pairs per d-chunk
    assert S % 512 == 0 and D % P == 0 and KL % P == 0 and KV <= 512

    nc = bacc.Bacc("TRN2", target_bir_lowering=False, debug=False)

    hS = nc.dram_tensor("hS", [S, D], bf16, kind="ExternalInput")
    hT = nc.dram_tensor("hT", [D, S], bf16, kind="ExternalInput")
    pkv = nc.dram_tensor("pkv", [S, KV], bf16, kind="ExternalInput")
    wqT = nc.dram_tensor("wqT", [D, D], bf16, kind="ExternalInput")
    wkT = nc.dram_tensor("wkT", [D, D], bf16, kind="ExternalInput")
    wvT = nc.dram_tensor("wvT", [D, D], bf16, kind="ExternalInput")
    bqs = nc.dram_tensor("bqs", [D], f32, kind="ExternalInput")
    bkr = nc.dram_tensor("bkr", [D], bf16, kind="ExternalInput")
    bvr = nc.dram_tensor("bvr", [D], bf16, kind="ExternalInput")
    msk = nc.dram_tensor("mask", [S], f32, kind="ExternalInput")
    out = nc.dram_tensor("out", [S, D], f32, kind="ExternalOutput")

    with tile.TileContext(nc) as tc:
        with (
            tc.tile_pool(name="persist", bufs=1) as persist,
            tc.tile_pool(name="wpool", bufs=1) as wpool,
            tc.tile_pool(name="hpool", bufs=3) as hpool,
            tc.tile_pool(name="spool", bufs=3) as spool,
            tc.tile_pool(name="qpool", bufs=2) as qpool,
            tc.tile_pool(name="cpool", bufs=2) as cpool,
        ):
            # ---------- setup: mask stats ----------
            with tc.tile_pool(name="psetup", bufs=1, space="PSUM") as psetup:
                mt = spool.tile([P, SC], f32, tag="mt")
                nc.sync.dma_start(out=mt[:], in_=msk.ap().rearrange("(c p) -> p c", p=P))
                m01 = spool.tile([P, SC], f32, tag="m01")
                nc.vector.tensor_scalar(m01[:], mt[:], -1.0, None, ALU.is_gt)
                lp = spool.tile([P, 1], f32, tag="lp")
                nc.vector.tensor_reduce(lp[:], m01[:], mybir.AxisListType.X, ALU.add)
                ones_col = persist.tile([P, 1], bf16, tag="onc")
                nc.vector.memset(ones_col[:], 1.0)
                ones_colf = spool.tile([P, 1], f32, tag="oncf")
                nc.vector.memset(ones_colf[:], 1.0)
                ones_row = spool.tile([1, P], f32, tag="onr")
                nc.vector.memset(ones_row[:], 1.0)
                lps = psetup.tile([1, 1], f32)
                nc.tensor.matmul(lps[:], lp[:], ones_colf[:], start=True, stop=True)
                lrec = spool.tile([1, 1], f32, tag="lrec")
                nc.vector.reciprocal(lrec[:], lps[:])
                inv = spool.tile([1, 1], f32, tag="inv")
                nc.scalar.activation(inv[:], lrec[:], AF.Sqrt)
                invps = psetup.tile([P, 1], f32)
                nc.tensor.matmul(invps[:], ones_row[:], inv[:], start=True, stop=True)
                invcol = spool.tile([P, 1], f32, tag="invcol")
                nc.vector.tensor_copy(invcol[:], invps[:])
                # mm = (mask > -1) / sqrt(L), per-s column layout [P, SC]
                mm_sb = persist.tile([P, SC], f32, tag="mmsb")
                nc.vector.tensor_scalar(mm_sb[:], m01[:], invcol[:], None, ALU.mult)
                mm_bf = persist.tile([P, SC], bf16, tag="mmbf")
                nc.vector.tensor_copy(mm_bf[:], mm_sb[:])

            # ---------- setup: biases ----------
            bq_sb = persist.tile([P, DC], f32, tag="bqsb")
            nc.sync.dma_start(out=bq_sb[:], in_=bqs.ap().rearrange("(m p) -> p m", p=P))
            bk_row = persist.tile([1, D], bf16, tag="bkrow")
            nc.sync.dma_start(out=bk_row[:], in_=bkr.ap()[None, :])
            bv_row = persist.tile([1, D], bf16, tag="bvrow")
            nc.sync.dma_start(out=bv_row[:], in_=bvr.ap()[None, :])

            # ---------- pkv resident ----------
            pkv_sb = wpool.tile([P, SC, KV], bf16, tag="pkvsb", name="pkvsb")
            nc.sync.dma_start(
                out=pkv_sb[:], in_=pkv.ap().rearrange("(c p) k -> p c k", p=P))

            # prefetch weights for phase 2 (used later; issue DMAs early)
            wk_sb = wpool.tile([P, DC, D], bf16, tag="wk", name="wksb")
            wv_sb = wpool.tile([P, DC, D], bf16, tag="wv", name="wvsb")
            for d in range(DC):
                nc.sync.dma_start(out=wk_sb[:, d, :],
                                  in_=wkT.ap()[P * d:P * (d + 1), :])
                nc.sync.dma_start(out=wv_sb[:, d, :],
                                  in_=wvT.ap()[P * d:P * (d + 1), :])

            # ---------- pass 0: cs = [csk | csv] = sum_s mm[s] * pkv[s, :] ----
            cs_sb = persist.tile([1, KV], bf16, tag="cssb")
            with tc.tile_pool(name="pcs", bufs=1, space="PSUM") as pcs:
                cs_ps = pcs.tile([1, KV], f32, name="csps")
                for s in range(SC):
                    nc.tensor.matmul(cs_ps[:], mm_bf[:, s:s + 1], pkv_sb[:, s, :],
                                     start=(s == 0), stop=(s == SC - 1))
                nc.vector.tensor_copy(cs_sb[:], cs_ps[:])

            # ---------- phase 1: hpT[d, :] = sum_s h[s, d] * pkvm[s, :] -------
            hp_sb = persist.tile([P, DC, KV], bf16, tag="hpsb")
            with tc.tile_pool(name="p1", bufs=1, space="PSUM") as p1:
                hp_ps = [p1.tile([P, KV], f32, tag=f"hp{d}", name=f"hpps{d}")
                         for d in range(DC)]
                for s in range(SC):
                    h_c = hpool.tile([P, D], bf16, tag="hc")
                    nc.sync.dma_start(out=h_c[:], in_=hS.ap()[P * s:P * (s + 1), :])
                    pkvm = spool.tile([P, KV], bf16, tag="pkvm")
                    nc.vector.tensor_scalar(pkvm[:], pkv_sb[:, s, :],
                                            mm_sb[:, s:s + 1], None, ALU.mult)
                    for d in range(DC):
                        nc.tensor.matmul(hp_ps[d][:], h_c[:, P * d:P * (d + 1)],
                                         pkvm[:], start=(s == 0), stop=(s == SC - 1))
                for d in range(DC):
                    nc.vector.tensor_copy(hp_sb[:, d, :], hp_ps[d][:])

            # ---------- phase 2k: kT[d, k] = sum_d' wkT[d', d] hpT[d', k] ----
            #            (+ bk[d] * csk[k])
            kt_sb = persist.tile([P, DC, KL], bf16, tag="ktsb")
            # ---------- phase 2v: v[k, d] = sum_d' hpT[d', K+k] wvT[d', d] ---
            #            (+ csv[k] * bv[d]) ; stored as vaug [k, dh+1] per head
            vaug = persist.tile([P, H * KC, DH + 1], bf16, tag="vaug")
            nc.vector.memset(vaug[:, :, DH:DH + 1], 1.0)
            D5 = (D + 511) // 512  # 512-wide column groups of D
            with tc.tile_pool(name="p2", bufs=1, space="PSUM") as p2:
                kt_ps = [p2.tile([P, 2 * KL], f32, tag=f"kt{j}", name=f"ktps{j}")
                         for j in range(DC // 2)]
                v_ps = [p2.tile([P, 512], f32, tag=f"v{j}", name=f"vps{j}")
                        for j in range(KC * D5)]
                for d in range(DC):
                    o = kt_ps[d // 2][:, (d % 2) * KL:(d % 2) * KL + KL]
                    for dp in range(DC):
                        nc.tensor.matmul(o, wk_sb[:, dp, P * d:P * (d + 1)],
                                         hp_sb[:, dp, 0:KL],
                                         start=(dp == 0), stop=False)
                    nc.tensor.matmul(o, bk_row[:, P * d:P * (d + 1)],
                                     cs_sb[:, 0:KL], start=False, stop=True)
                for kc in range(KC):
                    for j in range(D5):
                        o = v_ps[kc * D5 + j][:]
                        for dp in range(DC):
                            nc.tensor.matmul(
                                o, hp_sb[:, dp, KL + P * kc:KL + P * (kc + 1)],
                                wv_sb[:, dp, 512 * j:512 * (j + 1)],
                                start=(dp == 0), stop=False)
                        nc.tensor.matmul(
                            o, cs_sb[:, KL + P * kc:KL + P * (kc + 1)],
                            bv_row[:, 512 * j:512 * (j + 1)],
                            start=False, stop=True)
                for d in range(DC):
                    nc.vector.tensor_copy(
                        kt_sb[:, d, :], kt_ps[d // 2][:, (d % 2) * KL:(d % 2) * KL + KL])
                for i in range(H):
                    j, off = divmod(DH * i, 512)
                    for kc in range(KC):
                        nc.vector.tensor_copy(vaug[:, i * KC + kc, 0:DH],
                                              v_ps[kc * D5 + j][:, off:off + DH])

            # ---------- q + attention, per 512-group ----------
            wq_sb = wpool.tile([P, DC, D], bf16, tag="wq", name="wqsb")
            for d in range(DC):
                nc.sync.dma_start(out=wq_sb[:, d, :],
                                  in_=wqT.ap()[P * d:P * (d + 1), :])
            with (
                tc.tile_pool(name="pq", bufs=2, space="PSUM") as pq,
                tc.tile_pool(name="psc", bufs=2, space="PSUM") as psc,
                tc.tile_pool(name="pctx", bufs=2, space="PSUM") as pctx,
            ):
                for g in range(SG):
                    ht_g = hpool.tile([P, DC, 512], bf16, tag="ht")
                    for d in range(DC):
                        nc.sync.dma_start(
                            out=ht_g[:, d, :],
                            in_=hT.ap()[P * d:P * (d + 1), 512 * g:512 * (g + 1)])
                    qt_g = qpool.tile([P, DC, 512], bf16, tag="qt")
                    for mq in range(DC):
                        q_ps = pq.tile([P, 512], f32, tag="qps")
                        for d in range(DC):
                            nc.tensor.matmul(q_ps[:],
                                             wq_sb[:, d, P * mq:P * (mq + 1)],
                                             ht_g[:, d, :],
                                             start=(d == 0), stop=(d == DC - 1))
                        # q + bq/sqrt(DH); bias varies along partitions
                        nc.scalar.activation(qt_g[:, mq, :], q_ps[:], AF.Identity,
                                             bias=bq_sb[:, mq:mq + 1])
                    ctx_g = cpool.tile([P, 4, D], f32, tag="ctxg")
                    # software-pipelined head pairs: scores(j) ahead of ctx(j-1)
                    sc_tiles = {}
                    prob_tiles = {}

                    def emit_scores(j):
                        mq, hh = divmod(j, 2)
                        sc = psc.tile([P, KC, 512], f32, tag="sc")
                        po = DH * hh
                        for kc in range(KC):
                            nc.tensor.matmul(
                                sc[:, kc, :],
                                kt_sb[po:po + DH, mq, P * kc:P * (kc + 1)],
                                qt_g[po:po + DH, mq, :], start=True, stop=True)
                        sc_tiles[j] = sc

                    def emit_exp(j):
                        sc = sc_tiles.pop(j)
                        probT = spool.tile([P, KC, 512], bf16, tag="probT")
                        for kc in range(KC):
                            nc.scalar.activation(probT[:, kc, :], sc[:, kc, :],
                                                 AF.Exp)
                        prob_tiles[j] = probT

                    def emit_ctx(j):
                        i = j  # head index
                        probT = prob_tiles.pop(j)
                        ctx_ps = pctx.tile([P, 4, DH + 1], f32, tag="cx")
                        for c in range(4):
                            for kc in range(KC):
                                nc.tensor.matmul(
                                    ctx_ps[:, c, :],
                                    probT[:, kc, P * c:P * (c + 1)],
                                    vaug[:, i * KC + kc, :],
                                    start=(kc == 0), stop=(kc == KC - 1))
                        rec4 = spool.tile([P, 4, 1], f32, tag="rec4")
                        nc.vector.reciprocal(rec4[:], ctx_ps[:, :, DH:DH + 1])
                        nc.vector.tensor_tensor(
                            ctx_g[:, :, DH * i:DH * (i + 1)],
                            ctx_ps[:, :, 0:DH],
                            rec4[:].broadcast_to((P, 4, DH)), ALU.mult)

                    for j in range(H):
                        emit_scores(j)
                        emit_exp(j)
                        if j >= 1:
                            emit_ctx(j - 1)
                    emit_ctx(H - 1)
                    for c in range(4):
                        s0 = 512 * g + P * c
                        nc.sync.dma_start(out=out.ap()[s0:s0 + P, :],
                                          in_=ctx_g[:, c, :])

    nc.compile()
    return nc


_PROGRAM_CACHE = {}


def _get_program(S, D, KL):
    key = (S, D, KL)
    if key not in _PROGRAM_CACHE:
        _PROGRAM_CACHE[key] = build_program(S, D, KL)
    return _PROGRAM_CACHE[key]


def make_in_maps(hidden_states, attention_mask, Wq, bq, Wk, bk, Wv, bv,
                 proj_k, proj_v):
    """Host-side layout prep + batch sharding (1 sample per core)."""
    import ml_dtypes
    bf = ml_dtypes.bfloat16
    h = np.asarray(hidden_states, dtype=np.float32)
    Bn, S, D = h.shape
    scale = np.float32(1.0 / np.sqrt(DH))
    wqT = np.ascontiguousarray((np.asarray(Wq, np.float32) * scale).T).astype(bf)
    wkT = np.ascontiguousarray(np.asarray(Wk, np.float32).T).astype(bf)
    wvT = np.ascontiguousarray(np.asarray(Wv, np.float32).T).astype(bf)
    pkvn = np.concatenate([np.asarray(proj_k, np.float32)[:S],
                           np.asarray(proj_v, np.float32)[:S]], axis=1).astype(bf)
    bqn = (np.asarray(bq, np.float32) * scale).astype(np.float32)
    bkn = np.asarray(bk, np.float32).astype(bf)
    bvn = np.asarray(bv, np.float32).astype(bf)
    mask = np.asarray(attention_mask, np.float32).reshape(Bn, S)
    in_maps = []
    for b in range(Bn):
        hb = h[b]
        in_maps.append(dict(
            hS=np.ascontiguousarray(hb).astype(bf),
            hT=np.ascontiguousarray(hb.T).astype(bf),
            pkv=pkvn,
            wqT=wqT, wkT=wkT, wvT=wvT,
            bqs=bqn, bkr=bkn, bvr=bvn,
            mask=np.ascontiguousarray(mask[b]),
        ))
    return in_maps


def kernel(hidden_states, attention_mask, Wq, bq, Wk, bk, Wv, bv,
           proj_k, proj_v):
    h = np.asarray(hidden_states, dtype=np.float32)
    Bn, S, D = h.shape
    KL = np.asarray(proj_k).shape[1]
    nc = _get_program(S, D, KL)
    in_maps = make_in_maps(hidden_states, attention_mask, Wq, bq, Wk, bk,
                           Wv, bv, proj_k, proj_v)
    res = bass_utils.run_bass_kernel_spmd(nc, in_maps, core_ids=list(range(Bn)))
    return np.stack([res.results[b]["out"] for b in range(Bn)], axis=0)


def time_kernel(hidden_states, attention_mask, Wq, bq, Wk, bk, Wv, bv,
                proj_k, proj_v, k1=8, k2=40):
    """Estimate per-execution device time via pipelined-dispatch slope."""
    import time as _time
    import jax
    from jax.sharding import Mesh, PartitionSpec, NamedSharding
    from jax.experimental.shard_map import shard_map
    from concourse import bass2jax
    from concourse.bass2jax import _bass_exec_p, install_neuronx_cc_hook

    h = np.asarray(hidden_states, dtype=np.float32)
    Bn = h.shape[0]
    S, D = h.shape[1], h.shape[2]
    KL = np.asarray(proj_k).shape[1]
    nc = _get_program(S, D, KL)
    in_maps = make_in_maps(hidden_states, attention_mask, Wq, bq, Wk, bk,
                           Wv, bv, proj_k, proj_v)
    install_neuronx_cc_hook()
    partition_name = nc.partition_id_tensor.name if nc.partition_id_tensor else None
    in_names, out_names, out_avals = [], [], []
    for alloc in nc.m.functions[0].allocations:
        if not isinstance(alloc, mybir.MemoryLocationSet):
            continue
        name = alloc.memorylocations[0].name
        if alloc.kind == "ExternalInput":
            if name != partition_name:
                in_names.append(name)
        elif alloc.kind == "ExternalOutput":
            out_names.append(name)
            out_avals.append(jax.core.ShapedArray(
                tuple(alloc.tensor_shape), mybir.dt.np(alloc.dtype)))
    n_params = len(in_names)
    all_in = list(in_names) + list(out_names)
    if partition_name is not None:
        all_in.append(partition_name)

    def _body(*args):
        operands = list(args)
        if partition_name is not None:
            operands.append(bass2jax.partition_id_tensor())
        return tuple(_bass_exec_p.bind(
            *operands, out_avals=tuple(out_avals), in_names=tuple(all_in),
            out_names=tuple(out_names), lowering_input_output_aliases=(),
            sim_require_finite=True, sim_require_nnan=True, nc=nc))

    devices = jax.devices()[:Bn]
    mesh = Mesh(np.asarray(devices), ("core",))
    fn = jax.jit(shard_map(_body, mesh=mesh,
                           in_specs=(PartitionSpec("core"),) * (n_params + len(out_names)),
                           out_specs=(PartitionSpec("core"),) * len(out_names),
                           check_rep=False), keep_unused=True)
    sh = NamedSharding(mesh, PartitionSpec("core"))
    dev_in = [jax.device_put(
        np.concatenate([in_maps[c][nm] for c in range(Bn)], axis=0), sh)
        for nm in in_names]
    zer = [jax.device_put(np.zeros((Bn * a.shape[0], *a.shape[1:]), a.dtype), sh)
           for a in out_avals]
    outs = fn(*dev_in, *zer)
    jax.block_until_ready(outs)

    def run(k):
        t0 = _time.time()
        rs = [fn(*dev_in, *zer) for _ in range(k)]
        jax.block_until_ready(rs)
        return _time.time() - t0

    run(2)  # warm
    t_k1 = min(run(k1) for _ in range(2))
    t_k2 = min(run(k2) for _ in range(2))
    per_exec_s = (t_k2 - t_k1) / (k2 - k1)
    return per_exec_s * 1e9


# revision 3
# speedup vs baseline: 3.5171x; 1.4250x over previous
"""Trainium2 Bass kernel v2 for Linformer self-attention (ragged projection).

Per batch sample b (data-parallel over 8 cores), with K=256 << S=4096 the
k/v projections are reordered to project h down to K rows FIRST:

    L        = sum(mask > -1);  mm[s] = (mask[s] > -1) / sqrt(L)
    pkvm     = [pk | pv] * mm[:, None]                  # [S, 2K]
    hpT      = h.T @ pkvm                               # [D, 2K]   phase 1
    cs       = ones.T @ (pkv * mm)                      # [1, 2K]   (bias csum)
    kT       = Wk.T.T @ hpT[:, :K] + bk x csk           # [D, K]    phase 2k
    v        = hpT[:, K:].T @ Wv.T + csv x bv           # [K, D]    phase 2v
    qT       = (Wq.T/sqrt(DH)).T @ h.T + bq/sqrt(DH)    # [D, S]    per group
    per head i:  scT = kT_i.T @ qT_i    [K, 512]        # heads row-packed x2
                 probT = exp(scT)  (unnormalized, bf16)
                 ctx[s, 64i+j] = probT.T @ [v_i | 1]; normalize by col 64

This cuts matmul flops ~1.9x vs computing full [S,D] hk/hv. All matmul
inputs are bf16 (f32 PSUM accumulation); DMA volume is halved vs f32.
ctx is computed in [s, d] layout (probT chunks as the stationary operand)
so the softmax denominator lands per-partition -> cheap tensor_scalar
normalization and contiguous output DMA.
"""

import numpy as np

import concourse.mybir as mybir
import concourse.tile as tile
from concourse import bacc
from concourse import bass_utils

P = 128
f32 = mybir.dt.float32
bf16 = mybir.dt.bfloat16
AF = mybir.ActivationFunctionType
ALU = mybir.AluOpType

# Problem dims (nn_LinformerSelfAttention): B=8, S=4096, D=1024, H=16, K=256
B = 8
S_FULL = 4096
D_FULL = 1024
KL_FULL = 256
DH = 64


def build_program(S=S_FULL, D=D_FULL, KL=KL_FULL):
    SC = S // P           # s-chunks of 128 (32)
    SG = S // 512         # s-groups of 512 (8)
    DC = D // P           # d-chunks of 128 (8)
    KC = KL // P          # linformer-k chunks of 128 (2)
    KV = 2 * KL           # fused [pk|pv] width (512)
    H = D // DH           # heads (16)
    HP = H // 2           # head pairs per d-chunk
    assert S % 512 == 0 and D % P == 0 and KL % P == 0 and KV <= 512

    nc = bacc.Bacc("TRN2", target_bir_lowering=False, debug=False)

    hS = nc.dram_tensor("hS", [S, D], bf16, kind="ExternalInput")
    hT = nc.dram_tensor("hT", [D, S], bf16, kind="ExternalInput")
    pkv = nc.dram_tensor("pkv", [S, KV], bf16, kind="ExternalInput")
    wqT = nc.dram_tensor("wqT", [D, D], bf16, kind="ExternalInput")
    wkT = nc.dram_tensor("wkT", [D, D], bf16, kind="ExternalInput")
    wvT = nc.dram_tensor("wvT", [D, D], bf16, kind="ExternalInput")
    bqs = nc.dram_tensor("bqs", [D], f32, kind="ExternalInput")
    bkr = nc.dram_tensor("bkr", [D], bf16, kind="ExternalInput")
    bvr = nc.dram_tensor("bvr", [D], bf16, kind="ExternalInput")
    msk = nc.dram_tensor("mask", [S], f32, kind="ExternalInput")
    out = nc.dram_tensor("out", [S, D], f32, kind="ExternalOutput")

    with tile.TileContext(nc) as tc:
        with (
            tc.tile_pool(name="persist", bufs=1) as persist,
            tc.tile_pool(name="wpool", bufs=1) as wpool,
            tc.tile_pool(name="hpool", bufs=3) as hpool,
            tc.tile_pool(name="spool", bufs=3) as spool,
            tc.tile_pool(name="qpool", bufs=2) as qpool,
            tc.tile_pool(name="cpool", bufs=2) as cpool,
        ):
            # ---------- setup: mask stats ----------
            with tc.tile_pool(name="psetup", bufs=1, space="PSUM") as psetup:
                mt = spool.tile([P, SC], f32, tag="mt")
                nc.sync.dma_start(out=mt[:], in_=msk.ap().rearrange("(c p) -> p c", p=P))
                m01 = spool.tile([P, SC], f32, tag="m01")
                nc.vector.tensor_scalar(m01[:], mt[:], -1.0, None, ALU.is_gt)
                lp = spool.tile([P, 1], f32, tag="lp")
                nc.vector.tensor_reduce(lp[:], m01[:], mybir.AxisListType.X, ALU.add)
                ones_col = persist.tile([P, 1], bf16, tag="onc")
                nc.vector.memset(ones_col[:], 1.0)
                ones_colf = spool.tile([P, 1], f32, tag="oncf")
                nc.vector.memset(ones_colf[:], 1.0)
                ones_row = spool.tile([1, P], f32, tag="onr")
                nc.vector.memset(ones_row[:], 1.0)
                lps = psetup.tile([1, 1], f32)
                nc.tensor.matmul(lps[:], lp[:], ones_colf[:], start=True, stop=True)
                lrec = spool.tile([1, 1], f32, tag="lrec")
                nc.vector.reciprocal(lrec[:], lps[:])
                inv = spool.tile([1, 1], f32, tag="inv")
                nc.scalar.activation(inv[:], lrec[:], AF.Sqrt)
                invps = psetup.tile([P, 1], f32)
                nc.tensor.matmul(invps[:], ones_row[:], inv[:], start=True, stop=True)
                invcol = spool.tile([P, 1], f32, tag="invcol")
                nc.vector.tensor_copy(invcol[:], invps[:])
                # mm = (mask > -1) / sqrt(L), per-s column layout [P, SC]
                mm_sb = persist.tile([P, SC], f32, tag="mmsb")
                nc.vector.tensor_scalar(mm_sb[:], m01[:], invcol[:], None, ALU.mult)
                mm_bf = persist.tile([P, SC], bf16, tag="mmbf")
                nc.vector.tensor_copy(mm_bf[:], mm_sb[:])

            # ---------- setup: biases ----------
            bq_sb = persist.tile([P, DC], f32, tag="bqsb")
            nc.sync.dma_start(out=bq_sb[:], in_=bqs.ap().rearrange("(m p) -> p m", p=P))
            bk_row = persist.tile([1, D], bf16, tag="bkrow")
            nc.sync.dma_start(out=bk_row[:], in_=bkr.ap()[None, :])
            bv_row = persist.tile([1, D], bf16, tag="bvrow")
            nc.sync.dma_start(out=bv_row[:], in_=bvr.ap()[None, :])

            # ---------- phase 1: hpT[d, :] = sum_s h[s, d] * pkvm[s, :] -------
            # pkv chunks stream interleaved with hS chunks so the s-loop is
            # PE-bound from the start; pkv stays resident for the cs pass.
            pkv_sb = wpool.tile([P, SC, KV], bf16, tag="pkvsb", name="pkvsb")
            hp_sb = persist.tile([P, DC, KV], bf16, tag="hpsb")
            with tc.tile_pool(name="p1", bufs=1, space="PSUM") as p1:
                hp_ps = [p1.tile([P, KV], f32, tag=f"hp{d}", name=f"hpps{d}")
                         for d in range(DC)]
                for s in range(SC):
                    nc.sync.dma_start(out=pkv_sb[:, s, :],
                                      in_=pkv.ap()[P * s:P * (s + 1), :])
                    h_c = hpool.tile([P, D], bf16, tag="hc")
                    nc.sync.dma_start(out=h_c[:], in_=hS.ap()[P * s:P * (s + 1), :])
                    pkvm = spool.tile([P, KV], bf16, tag="pkvm")
                    nc.vector.tensor_scalar(pkvm[:], pkv_sb[:, s, :],
                                            mm_sb[:, s:s + 1], None, ALU.mult)
                    for d in range(DC):
                        nc.tensor.matmul(hp_ps[d][:], h_c[:, P * d:P * (d + 1)],
                                         pkvm[:], start=(s == 0), stop=(s == SC - 1))
                for d in range(DC):
                    nc.vector.tensor_copy(hp_sb[:, d, :], hp_ps[d][:])

            # phase-2/q weights: DMAs queue behind the hS stream, land during
            # the phase-1 compute.
            wk_sb = wpool.tile([P, DC, D], bf16, tag="wk", name="wksb")
            wv_sb = wpool.tile([P, DC, D], bf16, tag="wv", name="wvsb")
            for d in range(DC):
                nc.sync.dma_start(out=wk_sb[:, d, :],
                                  in_=wkT.ap()[P * d:P * (d + 1), :])
                nc.sync.dma_start(out=wv_sb[:, d, :],
                                  in_=wvT.ap()[P * d:P * (d + 1), :])

            # ---------- cs = [csk | csv] = sum_s mm[s] * pkv[s, :] ------------
            # (bias rank-1 csums; overlaps the hp PSUM->SBUF copies)
            cs_sb = persist.tile([1, KV], bf16, tag="cssb")
            with tc.tile_pool(name="pcs", bufs=1, space="PSUM") as pcs:
                cs_ps = pcs.tile([1, KV], f32, name="csps")
                for s in range(SC):
                    nc.tensor.matmul(cs_ps[:], mm_bf[:, s:s + 1], pkv_sb[:, s, :],
                                     start=(s == 0), stop=(s == SC - 1))
                nc.vector.tensor_copy(cs_sb[:], cs_ps[:])

            # ---------- phase 2k: kT[d, k] = sum_d' wkT[d', d] hpT[d', k] ----
            #            (+ bk[d] * csk[k])
            kt_sb = persist.tile([P, DC, KL], bf16, tag="ktsb")
            # ---------- phase 2v: v[k, d] = sum_d' hpT[d', K+k] wvT[d', d] ---
            #            (+ csv[k] * bv[d]) ; stored as vaug [k, dh+1] per head
            vaug = persist.tile([P, H * KC, DH + 1], bf16, tag="vaug")
            nc.vector.memset(vaug[:, :, DH:DH + 1], 1.0)
            D5 = (D + 511) // 512  # 512-wide column groups of D
            with tc.tile_pool(name="p2", bufs=1, space="PSUM") as p2:
                kt_ps = [p2.tile([P, 2 * KL], f32, tag=f"kt{j}", name=f"ktps{j}")
                         for j in range(DC // 2)]
                v_ps = [p2.tile([P, 512], f32, tag=f"v{j}", name=f"vps{j}")
                        for j in range(KC * D5)]
                for d in range(DC):
                    o = kt_ps[d // 2][:, (d % 2) * KL:(d % 2) * KL + KL]
                    for dp in range(DC):
                        nc.tensor.matmul(o, wk_sb[:, dp, P * d:P * (d + 1)],
                                         hp_sb[:, dp, 0:KL],
                                         start=(dp == 0), stop=False)
                    nc.tensor.matmul(o, bk_row[:, P * d:P * (d + 1)],
                                     cs_sb[:, 0:KL], start=False, stop=True)
                for kc in range(KC):
                    for j in range(D5):
                        o = v_ps[kc * D5 + j][:]
                        for dp in range(DC):
                            nc.tensor.matmul(
                                o, hp_sb[:, dp, KL + P * kc:KL + P * (kc + 1)],
                                wv_sb[:, dp, 512 * j:512 * (j + 1)],
                                start=(dp == 0), stop=False)
                        nc.tensor.matmul(
                            o, cs_sb[:, KL + P * kc:KL + P * (kc + 1)],
                            bv_row[:, 512 * j:512 * (j + 1)],
                            start=False, stop=True)
                for d in range(DC):
                    nc.vector.tensor_copy(
                        kt_sb[:, d, :], kt_ps[d // 2][:, (d % 2) * KL:(d % 2) * KL + KL])
                for i in range(H):
                    j, off = divmod(DH * i, 512)
                    for kc in range(KC):
                        nc.vector.tensor_copy(vaug[:, i * KC + kc, 0:DH],
                                              v_ps[kc * D5 + j][:, off:off + DH])

            # ---------- q + attention, per 512-group ----------
            wq_sb = wpool.tile([P, DC, D], bf16, tag="wq", name="wqsb")
            for d in range(DC):
                nc.sync.dma_start(out=wq_sb[:, d, :],
                                  in_=wqT.ap()[P * d:P * (d + 1), :])
            with (
                tc.tile_pool(name="pq", bufs=2, space="PSUM") as pq,
                tc.tile_pool(name="psc", bufs=2, space="PSUM") as psc,
                tc.tile_pool(name="pctx", bufs=2, space="PSUM") as pctx,
            ):
                for g in range(SG):
                    ht_g = hpool.tile([P, DC, 512], bf16, tag="ht")
                    for d in range(DC):
                        nc.sync.dma_start(
                            out=ht_g[:, d, :],
                            in_=hT.ap()[P * d:P * (d + 1), 512 * g:512 * (g + 1)])
                    qt_g = qpool.tile([P, DC, 512], bf16, tag="qt")
                    for mq in range(DC):
                        q_ps = pq.tile([P, 512], f32, tag="qps")
                        for d in range(DC):
                            nc.tensor.matmul(q_ps[:],
                                             wq_sb[:, d, P * mq:P * (mq + 1)],
                                             ht_g[:, d, :],
                                             start=(d == 0), stop=(d == DC - 1))
                        # q + bq/sqrt(DH); bias varies along partitions
                        nc.scalar.activation(qt_g[:, mq, :], q_ps[:], AF.Identity,
                                             bias=bq_sb[:, mq:mq + 1])
                    ctx_g = cpool.tile([P, 4, D], f32, tag="ctxg")
                    # software-pipelined head pairs: scores(j) ahead of ctx(j-1)
                    sc_tiles = {}
                    prob_tiles = {}

                    def emit_scores(j):
                        mq, hh = divmod(j, 2)
                        sc = psc.tile([P, KC, 512], f32, tag="sc")
                        po = DH * hh
                        for kc in range(KC):
                            nc.tensor.matmul(
                                sc[:, kc, :],
                                kt_sb[po:po + DH, mq, P * kc:P * (kc + 1)],
                                qt_g[po:po + DH, mq, :], start=True, stop=True)
                        sc_tiles[j] = sc

                    def emit_exp(j):
                        sc = sc_tiles.pop(j)
                        probT = spool.tile([P, KC, 512], bf16, tag="probT")
                        for kc in range(KC):
                            nc.scalar.activation(probT[:, kc, :], sc[:, kc, :],
                                                 AF.Exp)
                        prob_tiles[j] = probT

                    def emit_ctx(j):
                        i = j  # head index
                        probT = prob_tiles.pop(j)
                        ctx_ps = pctx.tile([P, 4, DH + 1], f32, tag="cx")
                        for c in range(4):
                            for kc in range(KC):
                                nc.tensor.matmul(
                                    ctx_ps[:, c, :],
                                    probT[:, kc, P * c:P * (c + 1)],
                                    vaug[:, i * KC + kc, :],
                                    start=(kc == 0), stop=(kc == KC - 1))
                        rec4 = spool.tile([P, 4, 1], f32, tag="rec4")
                        nc.vector.reciprocal(rec4[:], ctx_ps[:, :, DH:DH + 1])
                        nc.vector.tensor_tensor(
                            ctx_g[:, :, DH * i:DH * (i + 1)],
                            ctx_ps[:, :, 0:DH],
                            rec4[:].broadcast_to((P, 4, DH)), ALU.mult)

                    for j in range(H):
                        emit_scores(j)
                        emit_exp(j)
                        if j >= 1:
                            emit_ctx(j - 1)
                    emit_ctx(H - 1)
                    for c in range(4):
                        s0 = 512 * g + P * c
                        nc.sync.dma_start(out=out.ap()[s0:s0 + P, :],
                                          in_=ctx_g[:, c, :])

    nc.compile()
    return nc


_PROGRAM_CACHE = {}


def _get_program(S, D, KL):
    key = (S, D, KL)
    if key not in _PROGRAM_CACHE:
        _PROGRAM_CACHE[key] = build_program(S, D, KL)
    return _PROGRAM_CACHE[key]


def make_in_maps(hidden_states, attention_mask, Wq, bq, Wk, bk, Wv, bv,
                 proj_k, proj_v):
    """Host-side layout prep + batch sharding (1 sample per core)."""
    import ml_dtypes
    bf = ml_dtypes.bfloat16
    h = np.asarray(hidden_states, dtype=np.float32)
    Bn, S, D = h.shape
    scale = np.float32(1.0 / np.sqrt(DH))
    wqT = np.ascontiguousarray((np.asarray(Wq, np.float32) * scale).T).astype(bf)
    wkT = np.ascontiguousarray(np.asarray(Wk, np.float32).T).astype(bf)
    wvT = np.ascontiguousarray(np.asarray(Wv, np.float32).T).astype(bf)
    pkvn = np.concatenate([np.asarray(proj_k, np.float32)[:S],
                           np.asarray(proj_v, np.float32)[:S]], axis=1).astype(bf)
    bqn = (np.asarray(bq, np.float32) * scale).astype(np.float32)
    bkn = np.asarray(bk, np.float32).astype(bf)
    bvn = np.asarray(bv, np.float32).astype(bf)
    mask = np.asarray(attention_mask, np.float32).reshape(Bn, S)
    in_maps = []
    for b in range(Bn):
        hb = h[b]
        in_maps.append(dict(
            hS=np.ascontiguousarray(hb).astype(bf),
            hT=np.ascontiguousarray(hb.T).astype(bf),
            pkv=pkvn,
            wqT=wqT, wkT=wkT, wvT=wvT,
            bqs=bqn, bkr=bkn, bvr=bvn,
            mask=np.ascontiguousarray(mask[b]),
        ))
    return in_maps


def kernel(hidden_states, attention_mask, Wq, bq, Wk, bk, Wv, bv,
           proj_k, proj_v):
    h = np.asarray(hidden_states, dtype=np.float32)
    Bn, S, D = h.shape
    KL = np.asarray(proj_k).shape[1]
    nc = _get_program(S, D, KL)
    in_maps = make_in_maps(hidden_states, attention_mask, Wq, bq, Wk, bk,
                           Wv, bv, proj_k, proj_v)
    res = bass_utils.run_bass_kernel_spmd(nc, in_maps, core_ids=list(range(Bn)))
    return np.stack([res.results[b]["out"] for b in range(Bn)], axis=0)


def time_kernel(hidden_states, attention_mask, Wq, bq, Wk, bk, Wv, bv,
                proj_k, proj_v, k1=8, k2=40):
    """Estimate per-execution device time via pipelined-dispatch slope."""
    import time as _time
    import jax
    from jax.sharding import Mesh, PartitionSpec, NamedSharding
    from jax.experimental.shard_map import shard_map
    from concourse import bass2jax
    from concourse.bass2jax import _bass_exec_p, install_neuronx_cc_hook

    h = np.asarray(hidden_states, dtype=np.float32)
    Bn = h.shape[0]
    S, D = h.shape[1], h.shape[2]
    KL = np.asarray(proj_k).shape[1]
    nc = _get_program(S, D, KL)
    in_maps = make_in_maps(hidden_states, attention_mask, Wq, bq, Wk, bk,
                           Wv, bv, proj_k, proj_v)
    install_neuronx_cc_hook()
    partition_name = nc.partition_id_tensor.name if nc.partition_id_tensor else None
    in_names, out_names, out_avals = [], [], []
    for alloc in nc.m.functions[0].allocations:
        if not isinstance(alloc, mybir.MemoryLocationSet):
            continue
        name = alloc.memorylocations[0].name
        if alloc.kind == "ExternalInput":
            if name != partition_name:
                in_names.append(name)
        elif alloc.kind == "ExternalOutput":
            out_names.append(name)
            out_avals.append(jax.core.ShapedArray(
                tuple(alloc.tensor_shape), mybir.dt.np(alloc.dtype)))
    n_params = len(in_names)
    all_in = list(in_names) + list(out_names)
    if partition_name is not None:
        all_in.append(partition_name)

    def _body(*args):
        operands = list(args)
        if partition_name is not None:
            operands.append(bass2jax.partition_id_tensor())
        return tuple(_bass_exec_p.bind(
            *operands, out_avals=tuple(out_avals), in_names=tuple(all_in),
            out_names=tuple(out_names), lowering_input_output_aliases=(),
            sim_require_finite=True, sim_require_nnan=True, nc=nc))

    devices = jax.devices()[:Bn]
    mesh = Mesh(np.asarray(devices), ("core",))
    fn = jax.jit(shard_map(_body, mesh=mesh,
                           in_specs=(PartitionSpec("core"),) * (n_params + len(out_names)),
                           out_specs=(PartitionSpec("core"),) * len(out_names),
                           check_rep=False), keep_unused=True)
    sh = NamedSharding(mesh, PartitionSpec("core"))
    dev_in = [jax.device_put(
        np.concatenate([in_maps[c][nm] for c in range(Bn)], axis=0), sh)
        for nm in in_names]
    zer = [jax.device_put(np.zeros((Bn * a.shape[0], *a.shape[1:]), a.dtype), sh)
           for a in out_avals]
    outs = fn(*dev_in, *zer)
    jax.block_until_ready(outs)

    def run(k):
        t0 = _time.time()
        rs = [fn(*dev_in, *zer) for _ in range(k)]
        jax.block_until_ready(rs)
        return _time.time() - t0

    run(2)  # warm
    t_k1 = min(run(k1) for _ in range(2))
    t_k2 = min(run(k2) for _ in range(2))
    per_exec_s = (t_k2 - t_k1) / (k2 - k1)
    return per_exec_s * 1e9
